# revision 1
# baseline (speedup 1.0000x reference)
"""GCN (3x GCNConv+BN+ReLU, FC+sigmoid) on 8 Trainium2 NeuronCores.

Strategy (node-sharded, graph structure preprocessed on host):
  - Nodes sharded 8-ways (6250/core). Edges partitioned by destination core,
    sorted by destination block (128 dsts), padded to 128-edge chunks.
  - Per layer: GEMM y = s .* (h @ W) per-core (feature-major lhsT, node-major
    PSUM out, ACT epilogue applies per-node scale s and casts bf16), AllGather
    the bf16 node table, then dma_gather (SWDGE) pulls y[src] per edge chunk
    and a one-hot S matrix (built on DVE by comparing dst ids against an iota
    row) turns segment-sum into PSUM-accumulated matmuls: t[f,d] += msg.T @ S.
    Epilogue: h' = relu((t * s_dst) * a_f + cc_f) with BN folded into a/cc,
    written feature-major as the next layer's lhsT.
  - Gather-table indices are int16 (SWDGE limit), so the node table is split
    at row 32768: per block, edges are grouped into a "lo" stream (src <
    32768) and a "hi" stream, each gathered from its own table base.
  - Final FC + sigmoid on PE/ACT; output assembled on host.
"""
import os
import sys
sys.path.insert(0, "/opt/trn_rl_repo")

import numpy as np
import ml_dtypes

import concourse.bass as bass
import concourse.tile as tile
from concourse import mybir
from concourse.bass_utils import run_bass_kernel_spmd
from concourse.library_config import mlp as LIB_MLP
from concourse.tile_rust import add_dep_helper

BF16 = ml_dtypes.bfloat16
P = 128
NCORES = 8
BN_EPS = 1e-5
LO_LIMIT = 32768       # int16 index limit for the gather table
G_CALL = 8             # chunks (of 128 edges) per dma_gather call; 8*128 descs fit the 16KB SWDGE ring
PAD_DST = 200.0        # out-of-range dst id for padding edges

LAST_RESULTS = None    # test harness reads exec_time from here
LAST_NC = None         # built program, for cost-model timing in test.py

N_LAYERS = 3

# sweep knobs (env; defaults = shipped config)
GLO_BUFS = int(os.environ.get("SW_GLO", "3"))
GHI_BUFS = int(os.environ.get("SW_GHI", "2"))
ST_BUFS = int(os.environ.get("SW_STB", "3"))
TPS_BUFS = int(os.environ.get("SW_TPS", "4"))
YSB_BUFS = int(os.environ.get("SW_YSB", "3"))
N_QUEUES = int(os.environ.get("SW_NQ", "1"))
KB = int(os.environ.get("SW_KB", "8"))


def _split_multiwaits(nc):
    """This walrus build allows one sync-wait per instruction; move extras
    onto preceding same-engine NoOps."""
    n_new = 0
    for fn in nc.m.functions:
        for blk in fn.blocks:
            out = []
            changed = False
            for ins in list(blk.instructions):
                si = ins.sync_info
                if si is not None and len(si.on_wait) > 1:
                    waits = list(si.on_wait)
                    for w in waits[:-1]:
                        n_new += 1
                        out.append(mybir.InstNoOp(
                            name=f"I-mwsplit-{n_new}", engine=ins.engine,
                            sync_info=mybir.SyncInfo(on_wait=[w], on_update=[])))
                    si.on_wait = [waits[-1]]
                    changed = True
                out.append(ins)
            if changed:
                blk.instructions = out


def _prep_host(x, edge_index, n_nodes):
    """Shard + sort + pad the graph. Returns per-core tensors and the common
    (cross-core) block/chunk structure."""
    n_loc = n_nodes // NCORES
    n_blk = (n_loc + P - 1) // P

    src = np.concatenate([edge_index[0], np.arange(n_nodes, dtype=np.int64)])
    dst = np.concatenate([edge_index[1], np.arange(n_nodes, dtype=np.int64)])
    deg = np.bincount(dst, minlength=n_nodes).astype(np.float32)
    s = (1.0 / np.sqrt(np.maximum(deg, 1.0))).astype(np.float32)

    # per-core, per-block edge lists split by src range
    per_core = []
    for c in range(NCORES):
        mask = (dst >= c * n_loc) & (dst < (c + 1) * n_loc)
        cs, cd = src[mask], dst[mask] - c * n_loc
        blk = cd // P
        order = np.argsort(blk, kind="stable")
        cs, cd, blk = cs[order], cd[order], blk[order]
        lo_lists, hi_lists = [], []
        for b in range(n_blk):
            m = blk == b
            bs, bd = cs[m], cd[m] - b * P
            lo = bs < LO_LIMIT
            lo_lists.append((bs[lo], bd[lo]))
            hi_lists.append((bs[~lo] - LO_LIMIT, bd[~lo]))
        per_core.append((lo_lists, hi_lists))

    # common per-block chunk counts = max over cores (>=1 lo chunk per block)
    nlo = np.zeros(n_blk, np.int64)
    nhi = np.zeros(n_blk, np.int64)
    for c in range(NCORES):
        lo_lists, hi_lists = per_core[c]
        for b in range(n_blk):
            nlo[b] = max(nlo[b], (len(lo_lists[b][0]) + P - 1) // P)
            nhi[b] = max(nhi[b], (len(hi_lists[b][0]) + P - 1) // P)
    nlo = np.maximum(nlo, 1)
    NCL, NCH = int(nlo.sum()), int(nhi.sum())

    def pack(lists, n_chunks_per_blk, total_chunks):
        """Build gidx [128, total*8] int16 (16-wrap, x8 replicated) and
        dstid [128, total] bf16 for one stream."""
        gsrc = np.zeros(total_chunks * P, np.int64)
        gdst = np.full(total_chunks * P, PAD_DST, np.float32)
        pos = 0
        for b in range(len(n_chunks_per_blk)):
            bs, bd = lists[b]
            n = len(bs)
            cap = int(n_chunks_per_blk[b]) * P
            gsrc[pos:pos + n] = bs
            gdst[pos:pos + n] = bd
            pos += cap
        j = np.arange(total_chunks * P)
        gidx16 = np.zeros((16, total_chunks * 8), np.int16)
        gidx16[j % 16, j // 16] = gsrc
        gidx = np.tile(gidx16, (8, 1))
        dstid = np.zeros((P, total_chunks), dtype=BF16)
        dstid[j % P, j // P] = gdst.astype(BF16)
        return gidx, dstid

    cores = []
    for c in range(NCORES):
        lo_lists, hi_lists = per_core[c]
        gidx_lo, dstid_lo = pack(lo_lists, nlo, NCL)
        gidx_hi, dstid_hi = pack(hi_lists, nhi, NCH)

        s_loc = s[c * n_loc:(c + 1) * n_loc]
        s_col = np.zeros((P, n_blk), np.float32)
        for b in range(n_blk):
            nb = min(P, n_loc - b * P)
            s_col[:nb, b] = s_loc[b * P:b * P + nb]
        s_bcast = np.tile(s_loc[None, :], (P, 1)).astype(np.float32)

        xT = np.ascontiguousarray(x[c * n_loc:(c + 1) * n_loc].T).astype(BF16)
        cores.append(dict(xT=xT, gidx_lo=gidx_lo, gidx_hi=gidx_hi,
                          dstid_lo=dstid_lo, dstid_hi=dstid_hi,
                          s_col=s_col, s_bcast=s_bcast))
    return cores, nlo, nhi, NCL, NCH, n_loc, n_blk


def _build(n_nodes, n_loc, n_blk, nlo, nhi, NCL, NCH, feat, hid, bfc_val):
    nc = bass.Bass(num_swdge_queues=N_QUEUES)
    dt = mybir.dt
    NHI_ROWS = n_nodes - LO_LIMIT if n_nodes > LO_LIMIT else 0

    xT_in = nc.declare_dram_parameter("xT", [feat, n_loc], dt.bfloat16, isOutput=False)
    W_in = [nc.declare_dram_parameter(f"W{i}", [feat if i == 1 else hid, hid], dt.bfloat16, isOutput=False)
            for i in (1, 2, 3)]
    wfc_in = nc.declare_dram_parameter("wfc", [P, 2], dt.bfloat16, isOutput=False)
    aff_in = nc.declare_dram_parameter("aff", [P, 12], dt.float32, isOutput=False)
    s_col_in = nc.declare_dram_parameter("s_col", [P, n_blk], dt.float32, isOutput=False)
    s_bc_in = nc.declare_dram_parameter("s_bcast", [P, n_loc], dt.float32, isOutput=False)
    iota_in = nc.declare_dram_parameter("iota", [P, P], dt.bfloat16, isOutput=False)
    gidx_lo_in = nc.declare_dram_parameter("gidx_lo", [P, NCL * 8], dt.int16, isOutput=False)
    gidx_hi_in = nc.declare_dram_parameter("gidx_hi", [P, NCH * 8], dt.int16, isOutput=False)
    dstid_lo_in = nc.declare_dram_parameter("dstid_lo", [P, NCL], dt.bfloat16, isOutput=False)
    dstid_hi_in = nc.declare_dram_parameter("dstid_hi", [P, NCH], dt.bfloat16, isOutput=False)
    out_ext = nc.declare_dram_parameter("out", [1, n_loc], dt.float32, isOutput=True)

    y_loc = nc.dram_tensor("y_loc", [n_loc, hid], dt.bfloat16)
    y_full = nc.dram_tensor("y_full", [n_nodes, hid], dt.bfloat16, addr_space="Shared")

    lo_start = np.concatenate([[0], np.cumsum(nlo)])
    hi_start = np.concatenate([[0], np.cumsum(nhi)])
    calls_lo = [(c0, min(G_CALL, NCL - c0)) for c0 in range(0, NCL, G_CALL)]
    calls_hi = [(c0, min(G_CALL, NCH - c0)) for c0 in range(0, NCH, G_CALL)]
    batches_lo = [(c0, min(KB, NCL - c0)) for c0 in range(0, NCL, KB)]
    batches_hi = [(c0, min(KB, NCH - c0)) for c0 in range(0, NCH, KB)]

    with tile.TileContext(nc) as tc:
        with tc.tile_pool(name="const", bufs=1) as cpool, \
             tc.tile_pool(name="ht", bufs=2) as hpool, \
             tc.tile_pool(name="glo", bufs=GLO_BUFS) as glo_pool, \
             tc.tile_pool(name="ghi", bufs=GHI_BUFS) as ghi_pool, \
             tc.tile_pool(name="work", bufs=YSB_BUFS) as wpool, \
             tc.tile_pool(name="ep", bufs=2) as epool, \
             tc.tile_pool(name="stp", bufs=ST_BUFS) as spool, \
             tc.tile_pool(name="psy", bufs=2, space="PSUM") as psy, \
             tc.tile_pool(name="pst", bufs=TPS_BUFS, space="PSUM") as pst, \
             tc.tile_pool(name="psf", bufs=1, space="PSUM") as psf:

            lib_inst = nc.gpsimd.load_library(LIB_MLP)

            # to_reg leaks a Pool register per call; cache per distinct count
            _nreg = {}

            def nidx_reg(n):
                if n not in _nreg:
                    _nreg[n] = nc.gpsimd.to_reg(n)
                return _nreg[n]

            # ---- constants ----
            iota = cpool.tile([P, P], dt.bfloat16)
            nc.sync.dma_start(out=iota[:], in_=iota_in[:, :])
            gidx_lo = cpool.tile([P, NCL * 8], dt.int16)
            nc.sync.dma_start(out=gidx_lo[:], in_=gidx_lo_in[:, :])
            gidx_hi = cpool.tile([P, NCH * 8], dt.int16)
            nc.sync.dma_start(out=gidx_hi[:], in_=gidx_hi_in[:, :])
            dstid_lo = cpool.tile([P, NCL], dt.bfloat16)
            nc.sync.dma_start(out=dstid_lo[:], in_=dstid_lo_in[:, :])
            dstid_hi = cpool.tile([P, NCH], dt.bfloat16)
            nc.sync.dma_start(out=dstid_hi[:], in_=dstid_hi_in[:, :])
            s_col = cpool.tile([P, n_blk], dt.float32)
            nc.sync.dma_start(out=s_col[:], in_=s_col_in[:, :])
            s_bc = cpool.tile([P, n_loc], dt.float32)
            nc.sync.dma_start(out=s_bc[:], in_=s_bc_in[:, :])
            aff = cpool.tile([P, 12], dt.float32)
            nc.sync.dma_start(out=aff[:], in_=aff_in[:, :])
            wfc = cpool.tile([P, 2], dt.bfloat16)
            nc.sync.dma_start(out=wfc[:], in_=wfc_in[:, :])
            Ws = []
            for i in range(3):
                wlo = cpool.tile([P, hid], dt.bfloat16, tag=f"w{i}lo")
                nc.sync.dma_start(out=wlo[:], in_=W_in[i][0:P, :])
                whi = cpool.tile([P, hid], dt.bfloat16, tag=f"w{i}hi")
                nc.sync.dma_start(out=whi[:], in_=W_in[i][P:2 * P, :])
                Ws.append((wlo, whi))

            # ---- initial h_T = x_T (bf16, feature-major halves) ----
            h_lo = hpool.tile([P, n_loc], dt.bfloat16, tag="h0")
            nc.sync.dma_start(out=h_lo[:], in_=xT_in[0:P, :])
            h_hi = hpool.tile([P, n_loc], dt.bfloat16, tag="h1")
            nc.sync.dma_start(out=h_hi[:], in_=xT_in[P:2 * P, :])

            for layer in range(N_LAYERS):
                wlo, whi = Ws[layer]
                # ---- GEMM: y = s .* (h @ W), write bf16 table ----
                for b in range(n_blk):
                    bs = b * P
                    nb = min(P, n_loc - bs)
                    ps = psy.tile([P, hid], dt.float32, tag="ypsum")
                    nc.tensor.matmul(out=ps[:nb, :], lhsT=h_lo[:, bs:bs + nb],
                                     rhs=wlo[:, :], start=True, stop=False)
                    nc.tensor.matmul(out=ps[:nb, :], lhsT=h_hi[:, bs:bs + nb],
                                     rhs=whi[:, :], start=False, stop=True)
                    ysb = wpool.tile([P, hid], dt.bfloat16, tag="ysb")
                    nc.scalar.activation(ysb[:nb, :], ps[:nb, :],
                                         mybir.ActivationFunctionType.Copy,
                                         scale=s_col[:nb, b:b + 1])
                    nc.sync.dma_start(out=y_loc[bs:bs + nb, :], in_=ysb[:nb, :])

                # ---- AllGather the table ----
                nc.gpsimd.collective_compute(
                    "AllGather", mybir.AluOpType.bypass,
                    replica_groups=[list(range(NCORES))],
                    ins=[y_loc[:, :]], outs=[y_full[:, :]],
                )

                # ---- gathers ----
                def emit_gathers(calls, gpool, gidx, table_ap, tagn, qn=0):
                    tiles = []
                    for (c0, cnt) in calls:
                        gt = gpool.tile([P, G_CALL * hid], dt.bfloat16, tag=tagn)
                        g = nc.gpsimd.dma_gather(
                            out_ap=gt[:, :cnt * hid].rearrange("p (g f) -> p g f", g=cnt),
                            in_ap=table_ap,
                            idxs_ap=gidx[:, c0 * 8:(c0 + cnt) * 8],
                            num_idxs=cnt * P,
                            num_idxs_reg=nidx_reg(cnt * P),
                            elem_size=hid,
                            queue_num=qn % N_QUEUES,
                        )
                        add_dep_helper(g.ins, lib_inst.ins, sync=False, reason="lib first")
                        tiles.append(gt)
                    return tiles

                gt_lo = emit_gathers(calls_lo, glo_pool, gidx_lo,
                                     y_full[0:min(LO_LIMIT, n_nodes), :], "glo")
                gt_hi = emit_gathers(calls_hi, ghi_pool, gidx_hi,
                                     y_full[LO_LIMIT:n_nodes, :] if NHI_ROWS else y_full[0:1, :],
                                     "ghi", qn=1) if NCH else []

                # ---- batched one-hot compares ----
                def emit_compares(batches, dstid, tagn):
                    tiles = []
                    for (c0, cnt) in batches:
                        st = spool.tile([P, KB * P], dt.bfloat16, tag=tagn)
                        nc.vector.tensor_tensor(
                            out=st[:, :cnt * P].rearrange("p (c d) -> p c d", c=cnt),
                            in0=dstid[:, c0:c0 + cnt, None].to_broadcast([P, cnt, P]),
                            in1=iota[:, None, :].to_broadcast([P, cnt, P]),
                            op=mybir.AluOpType.is_equal,
                        )
                        tiles.append(st)
                    return tiles

                st_lo = emit_compares(batches_lo, dstid_lo, "stlo")
                st_hi = emit_compares(batches_hi, dstid_hi, "sthi") if NCH else []

                # ---- per-block accumulate + epilogue ----
                h_lo_new = hpool.tile([P, n_loc], dt.bfloat16, tag="h0")
                h_hi_new = hpool.tile([P, n_loc], dt.bfloat16, tag="h1")
                for b in range(n_blk):
                    bs = b * P
                    nb = min(P, n_loc - bs)
                    seq = [(gt_lo, st_lo, c) for c in range(lo_start[b], lo_start[b + 1])] + \
                          [(gt_hi, st_hi, c) for c in range(hi_start[b], hi_start[b + 1])]
                    for h, h_new in ((0, h_lo_new), (1, h_hi_new)):
                        ps = pst.tile([P, P], dt.float32, tag="tpsum")
                        for i, (gts, sts, c) in enumerate(seq):
                            gt = gts[c // G_CALL]
                            goff = (c % G_CALL) * hid + h * P
                            st = sts[c // KB]
                            soff = (c % KB) * P
                            nc.tensor.matmul(
                                out=ps[:, :], lhsT=gt[:, goff:goff + P],
                                rhs=st[:, soff:soff + P],
                                start=(i == 0), stop=(i == len(seq) - 1),
                            )
                        tmp = epool.tile([P, P], dt.float32, tag="eptmp")
                        nc.vector.tensor_tensor(out=tmp[:, :nb], in0=ps[:, :nb],
                                                in1=s_bc[:, bs:bs + nb],
                                                op=mybir.AluOpType.mult)
                        a_ap = aff[:, 4 * layer + h:4 * layer + h + 1]
                        cc_ap = aff[:, 4 * layer + 2 + h:4 * layer + 3 + h]
                        nc.scalar.activation(h_new[:, bs:bs + nb], tmp[:, :nb],
                                             mybir.ActivationFunctionType.Relu,
                                             bias=cc_ap, scale=a_ap)
                h_lo, h_hi = h_lo_new, h_hi_new

            # ---- FC + sigmoid ----
            osb = cpool.tile([1, n_loc], dt.float32)
            for t0 in range(0, n_loc, 512):
                w = min(512, n_loc - t0)
                ps = psf.tile([1, 512], dt.float32, tag="fcps")
                nc.tensor.matmul(out=ps[:1, :w], lhsT=wfc[:, 0:1],
                                 rhs=h_lo[:, t0:t0 + w], start=True, stop=False)
                nc.tensor.matmul(out=ps[:1, :w], lhsT=wfc[:, 1:2],
                                 rhs=h_hi[:, t0:t0 + w], start=False, stop=True)
                nc.scalar.activation(osb[:1, t0:t0 + w], ps[:1, :w],
                                     mybir.ActivationFunctionType.Sigmoid,
                                     bias=float(bfc_val), scale=1.0)
            nc.sync.dma_start(out=out_ext[:, :], in_=osb[:])

    mybir.codegen_inst_isa_subclasses(nc)
    _split_multiwaits(nc)
    return nc


def kernel(**inputs):
    global LAST_RESULTS, LAST_NC
    x = np.asarray(inputs["x"], dtype=np.float32)
    edge_index = np.asarray(inputs["edge_index"])
    n_nodes, feat = x.shape
    hid = np.asarray(inputs["W1"]).shape[1]

    cores, nlo, nhi, NCL, NCH, n_loc, n_blk = _prep_host(x, edge_index, n_nodes)

    # BN affine folding: z = (agg + b - m) * a + be,  a = g * rsqrt(v + eps)
    aff = np.zeros((P, 12), np.float32)
    for i in (1, 2, 3):
        g = np.asarray(inputs[f"g{i}"], np.float32)
        be = np.asarray(inputs[f"be{i}"], np.float32)
        m = np.asarray(inputs[f"m{i}"], np.float32)
        v = np.asarray(inputs[f"v{i}"], np.float32)
        b = np.asarray(inputs[f"b{i}"], np.float32)
        a = g / np.sqrt(v + BN_EPS)
        cc = (b - m) * a + be
        L = i - 1
        aff[:, 4 * L + 0] = a[0:P]
        aff[:, 4 * L + 1] = a[P:2 * P]
        aff[:, 4 * L + 2] = cc[0:P]
        aff[:, 4 * L + 3] = cc[P:2 * P]

    wfc_np = np.zeros((P, 2), dtype=BF16)
    Wfc = np.asarray(inputs["Wfc"], np.float32)
    wfc_np[:, 0] = Wfc[0:P, 0].astype(BF16)
    wfc_np[:, 1] = Wfc[P:2 * P, 0].astype(BF16)
    bfc_val = float(np.asarray(inputs["bfc"]).reshape(-1)[0])
    iota_np = np.tile(np.arange(P, dtype=np.float32).astype(BF16)[None, :], (P, 1))

    nc = _build(n_nodes, n_loc, n_blk, nlo, nhi, NCL, NCH, feat, hid, bfc_val)

    in_maps = []
    for c in range(NCORES):
        d = cores[c]
        in_maps.append({
            "xT": d["xT"],
            "W1": np.asarray(inputs["W1"], np.float32).astype(BF16),
            "W2": np.asarray(inputs["W2"], np.float32).astype(BF16),
            "W3": np.asarray(inputs["W3"], np.float32).astype(BF16),
            "wfc": wfc_np, "aff": aff,
            "s_col": d["s_col"], "s_bcast": d["s_bcast"], "iota": iota_np,
            "gidx_lo": d["gidx_lo"], "gidx_hi": d["gidx_hi"],
            "dstid_lo": d["dstid_lo"], "dstid_hi": d["dstid_hi"],
        })

    res = run_bass_kernel_spmd(nc, in_maps, core_ids=list(range(NCORES)))
    LAST_RESULTS = res
    globals()["LAST_NC"] = nc
    out = np.concatenate([res.results[c]["out"].reshape(-1) for c in range(NCORES)])
    return out.reshape(-1, 1).astype(np.float32)



# revision 46
# speedup vs baseline: 1.6318x; 1.6318x over previous
"""GCN (3x GCNConv+BN+ReLU, FC+sigmoid) on 8 Trainium2 NeuronCores.

Strategy (node-sharded, graph structure preprocessed on host):
  - Nodes sharded 8-ways (6250/core). Edges partitioned by destination core,
    sorted by destination block (128 dsts), padded to 128-edge chunks.
  - Layer 1 exploits linearity (agg(x@W1) == agg(x)@W1): the host ships the
    full pre-scaled input table x_tbl = s .* x (bf16, identical per core), so
    layer 1 has NO GEMM and NO AllGather — it aggregates raw x rows and
    applies W1 per destination block afterwards (PSUM -> bf16 copy, then
    2x2 half matmuls), saving one full-table collective.
  - Layers 2-3: GEMM y = s .* (h @ W) per-core is emitted inline (delayed by
    GEMM_DLY blocks to avoid PE head-of-line stalls) inside the previous
    layer's block loop; the fp8e4 node table (halves the collective's modeled
    bytes; rel err stays ~2e-3) is AllGathered, then dma_gather (SWDGE) pulls
    y[src] per edge chunk and a one-hot S matrix (DVE is_equal against an
    iota row) turns segment-sum into PSUM-accumulated matmuls:
    t[f,d] += msg.T @ S. Epilogue: h' = relu((t * s_dst) * a_f + cc_f) with
    BN folded into a/cc, written feature-major as the next layer's lhsT.
  - Gather-table indices are int16 (SWDGE limit), so the node table is split
    at row 32768 into "lo"/"hi" streams. Gather calls and compare batches for
    the two streams are emitted interleaved in block-consumption order — each
    block's PSUM accumulation needs both streams, so emitting all lo calls
    first would stall every block on the first hi gather (~100us).
  - Final FC + sigmoid on PE/ACT; output assembled on host.
"""
import os
import sys
sys.path.insert(0, "/opt/trn_rl_repo")

import numpy as np
import ml_dtypes

import concourse.bass as bass
import concourse.tile as tile
from concourse import mybir
from concourse.bass_utils import run_bass_kernel_spmd
from concourse.library_config import mlp as LIB_MLP
from concourse.tile_rust import add_dep_helper

BF16 = ml_dtypes.bfloat16
P = 128
NCORES = 8
BN_EPS = 1e-5
LO_LIMIT = 32768       # int16 index limit for the gather table
PAD_DST = 200.0        # out-of-range dst id for padding edges

LAST_RESULTS = None    # test harness reads exec_time from here
LAST_NC = None         # built program, for cost-model timing in test.py

N_LAYERS = 3

# sweep knobs (env; defaults = shipped config)
GLO_BUFS = int(os.environ.get("SW_GLO", "5"))
GHI_BUFS = int(os.environ.get("SW_GHI", "4"))
ST_BUFS = int(os.environ.get("SW_STB", "5"))
FP8_TBL = int(os.environ.get("SW_FP8", "1"))
XP_MOD = int(os.environ.get("SW_XP", "2"))  # expand dstid via ACT for batches with idx % XP_MOD != 0
TPS_BUFS = int(os.environ.get("SW_TPS", "3"))
TW_BUFS = int(os.environ.get("SW_TW", "2"))
YSB_BUFS = int(os.environ.get("SW_YSB", "3"))
N_QUEUES = int(os.environ.get("SW_NQ", "1"))
KB = int(os.environ.get("SW_KB", "16"))
GEMM_DLY = int(os.environ.get("SW_GD", "9"))
G_CALL = int(os.environ.get("SW_GC", "8"))  # chunks (of 128 edges) per dma_gather call; 8*128 descs fit the 16KB SWDGE ring


def _split_multiwaits(nc):
    """This walrus build allows one sync-wait per instruction; move extras
    onto preceding same-engine NoOps."""
    n_new = 0
    for fn in nc.m.functions:
        for blk in fn.blocks:
            out = []
            changed = False
            for ins in list(blk.instructions):
                si = ins.sync_info
                if si is not None and len(si.on_wait) > 1:
                    waits = list(si.on_wait)
                    for w in waits[:-1]:
                        n_new += 1
                        out.append(mybir.InstNoOp(
                            name=f"I-mwsplit-{n_new}", engine=ins.engine,
                            sync_info=mybir.SyncInfo(on_wait=[w], on_update=[])))
                    si.on_wait = [waits[-1]]
                    changed = True
                out.append(ins)
            if changed:
                blk.instructions = out


def _prep_host(x, edge_index, n_nodes):
    """Shard + sort + pad the graph. Returns per-core tensors and the common
    (cross-core) block/chunk structure."""
    n_loc = n_nodes // NCORES
    n_blk = (n_loc + P - 1) // P

    src = np.concatenate([edge_index[0], np.arange(n_nodes, dtype=np.int64)])
    dst = np.concatenate([edge_index[1], np.arange(n_nodes, dtype=np.int64)])
    deg = np.bincount(dst, minlength=n_nodes).astype(np.float32)
    s = (1.0 / np.sqrt(np.maximum(deg, 1.0))).astype(np.float32)

    # per-core, per-block edge lists split by src range
    per_core = []
    for c in range(NCORES):
        mask = (dst >= c * n_loc) & (dst < (c + 1) * n_loc)
        cs, cd = src[mask], dst[mask] - c * n_loc
        blk = cd // P
        order = np.argsort(blk, kind="stable")
        cs, cd, blk = cs[order], cd[order], blk[order]
        lo_lists, hi_lists = [], []
        for b in range(n_blk):
            m = blk == b
            bs, bd = cs[m], cd[m] - b * P
            lo = bs < LO_LIMIT
            lo_lists.append((bs[lo], bd[lo]))
            hi_lists.append((bs[~lo] - LO_LIMIT, bd[~lo]))
        per_core.append((lo_lists, hi_lists))

    # common per-block chunk counts = max over cores (>=1 lo chunk per block)
    nlo = np.zeros(n_blk, np.int64)
    nhi = np.zeros(n_blk, np.int64)
    for c in range(NCORES):
        lo_lists, hi_lists = per_core[c]
        for b in range(n_blk):
            nlo[b] = max(nlo[b], (len(lo_lists[b][0]) + P - 1) // P)
            nhi[b] = max(nhi[b], (len(hi_lists[b][0]) + P - 1) // P)
    nlo = np.maximum(nlo, 1)
    NCL, NCH = int(nlo.sum()), int(nhi.sum())

    def pack(lists, n_chunks_per_blk, total_chunks):
        """Build gidx [128, total*8] int16 (16-wrap, x8 replicated) and
        dstid [128, total] bf16 for one stream."""
        gsrc = np.zeros(total_chunks * P, np.int64)
        gdst = np.full(total_chunks * P, PAD_DST, np.float32)
        pos = 0
        for b in range(len(n_chunks_per_blk)):
            bs, bd = lists[b]
            n = len(bs)
            cap = int(n_chunks_per_blk[b]) * P
            gsrc[pos:pos + n] = bs
            gdst[pos:pos + n] = bd
            pos += cap
        j = np.arange(total_chunks * P)
        gidx16 = np.zeros((16, total_chunks * 8), np.int16)
        gidx16[j % 16, j // 16] = gsrc
        gidx = np.tile(gidx16, (8, 1))
        dstid = np.zeros((P, total_chunks), dtype=BF16)
        dstid[j % P, j // P] = gdst.astype(BF16)
        return gidx, dstid

    cores = []
    for c in range(NCORES):
        lo_lists, hi_lists = per_core[c]
        gidx_lo, dstid_lo = pack(lo_lists, nlo, NCL)
        gidx_hi, dstid_hi = pack(hi_lists, nhi, NCH)

        s_loc = s[c * n_loc:(c + 1) * n_loc]
        s_col = np.zeros((P, n_blk), np.float32)
        for b in range(n_blk):
            nb = min(P, n_loc - b * P)
            s_col[:nb, b] = s_loc[b * P:b * P + nb]
        s_bcast = np.tile(s_loc[None, :], (P, 1)).astype(BF16)

        cores.append(dict(gidx_lo=gidx_lo, gidx_hi=gidx_hi,
                          dstid_lo=dstid_lo, dstid_hi=dstid_hi,
                          s_col=s_col, s_bcast=s_bcast))
    return cores, nlo, nhi, NCL, NCH, n_loc, n_blk, s


def _build(n_nodes, n_loc, n_blk, nlo, nhi, NCL, NCH, feat, hid, bfc_val):
    nc = bass.Bass(num_swdge_queues=N_QUEUES,
                   dynamic_dma_scratch_size=max(16384, G_CALL * P * 16))
    dt = mybir.dt
    NHI_ROWS = n_nodes - LO_LIMIT if n_nodes > LO_LIMIT else 0

    # full pre-scaled input table: x_tbl[n] = s_n * x[n]  (layer-1 aggregates this
    # directly — agg(x@W1) == agg(x)@W1 — so no GEMM/AllGather before layer 1)
    x_tbl_in = nc.declare_dram_parameter("x_tbl", [n_nodes, feat], dt.bfloat16, isOutput=False)
    W_in = [nc.declare_dram_parameter(f"W{i}", [feat if i == 1 else hid, hid], dt.bfloat16, isOutput=False)
            for i in (1, 2, 3)]
    wfc_in = nc.declare_dram_parameter("wfc", [P, 2], dt.bfloat16, isOutput=False)
    aff_in = nc.declare_dram_parameter("aff", [P, 12], dt.float32, isOutput=False)
    s_col_in = nc.declare_dram_parameter("s_col", [P, n_blk], dt.float32, isOutput=False)
    s_bc_in = nc.declare_dram_parameter("s_bcast", [P, n_loc], dt.bfloat16, isOutput=False)
    iota_in = nc.declare_dram_parameter("iota", [P, P], dt.bfloat16, isOutput=False)
    gidx_lo_in = nc.declare_dram_parameter("gidx_lo", [P, NCL * 8], dt.int16, isOutput=False)
    gidx_hi_in = nc.declare_dram_parameter("gidx_hi", [P, NCH * 8], dt.int16, isOutput=False)
    dstid_lo_in = nc.declare_dram_parameter("dstid_lo", [P, NCL], dt.bfloat16, isOutput=False)
    dstid_hi_in = nc.declare_dram_parameter("dstid_hi", [P, NCH], dt.bfloat16, isOutput=False)
    out_ext = nc.declare_dram_parameter("out", [1, n_loc], dt.float32, isOutput=True)

    tbl_dt = dt.float8e4 if FP8_TBL else dt.bfloat16
    y_loc = nc.dram_tensor("y_loc", [n_loc, hid], tbl_dt)
    y_full = nc.dram_tensor("y_full", [n_nodes, hid], tbl_dt, addr_space="Shared")

    lo_start = np.concatenate([[0], np.cumsum(nlo)])
    hi_start = np.concatenate([[0], np.cumsum(nhi)])
    calls_lo = [(c0, min(G_CALL, NCL - c0)) for c0 in range(0, NCL, G_CALL)]
    calls_hi = [(c0, min(G_CALL, NCH - c0)) for c0 in range(0, NCH, G_CALL)]
    batches_lo = [(c0, min(KB, NCL - c0)) for c0 in range(0, NCL, KB)]
    batches_hi = [(c0, min(KB, NCH - c0)) for c0 in range(0, NCH, KB)]

    with tile.TileContext(nc) as tc:
        with tc.tile_pool(name="const", bufs=1) as cpool, \
             tc.tile_pool(name="ht", bufs=2) as hpool, \
             tc.tile_pool(name="glo", bufs=GLO_BUFS) as glo_pool, \
             tc.tile_pool(name="ghi", bufs=GHI_BUFS) as ghi_pool, \
             tc.tile_pool(name="work", bufs=YSB_BUFS) as wpool, \
             tc.tile_pool(name="ep", bufs=2) as epool, \
             tc.tile_pool(name="stp", bufs=ST_BUFS) as spool, \
             tc.tile_pool(name="dstx", bufs=3) as xpool, \
             tc.tile_pool(name="psy", bufs=2, space="PSUM") as psy, \
             tc.tile_pool(name="pst", bufs=TPS_BUFS, space="PSUM") as pst, \
             tc.tile_pool(name="ptw", bufs=TW_BUFS, space="PSUM") as ptw, \
             tc.tile_pool(name="psf", bufs=1, space="PSUM") as psf:

            lib_inst = nc.gpsimd.load_library(LIB_MLP)

            # to_reg leaks a Pool register per call; cache per distinct count
            _nreg = {}

            def nidx_reg(n):
                if n not in _nreg:
                    _nreg[n] = nc.gpsimd.to_reg(n)
                return _nreg[n]

            # ---- constants ----
            iota = cpool.tile([P, P], dt.bfloat16)
            nc.sync.dma_start(out=iota[:], in_=iota_in[:, :])
            gidx_lo = cpool.tile([P, NCL * 8], dt.int16)
            nc.sync.dma_start(out=gidx_lo[:], in_=gidx_lo_in[:, :])
            gidx_hi = cpool.tile([P, NCH * 8], dt.int16)
            nc.sync.dma_start(out=gidx_hi[:], in_=gidx_hi_in[:, :])
            dstid_lo = cpool.tile([P, NCL], dt.bfloat16)
            nc.sync.dma_start(out=dstid_lo[:], in_=dstid_lo_in[:, :])
            dstid_hi = cpool.tile([P, NCH], dt.bfloat16)
            nc.sync.dma_start(out=dstid_hi[:], in_=dstid_hi_in[:, :])
            s_col = cpool.tile([P, n_blk], dt.float32)
            nc.sync.dma_start(out=s_col[:], in_=s_col_in[:, :])
            s_bc = cpool.tile([P, n_loc], dt.bfloat16)
            nc.sync.dma_start(out=s_bc[:], in_=s_bc_in[:, :])
            aff = cpool.tile([P, 12], dt.float32)
            nc.sync.dma_start(out=aff[:], in_=aff_in[:, :])
            wfc = cpool.tile([P, 2], dt.bfloat16)
            nc.sync.dma_start(out=wfc[:], in_=wfc_in[:, :])
            Ws = []
            for i in range(3):
                wlo = cpool.tile([P, hid], dt.bfloat16, tag=f"w{i}lo")
                nc.sync.dma_start(out=wlo[:], in_=W_in[i][0:P, :])
                whi = cpool.tile([P, hid], dt.bfloat16, tag=f"w{i}hi")
                nc.sync.dma_start(out=whi[:], in_=W_in[i][P:2 * P, :])
                Ws.append((wlo, whi))

            h_lo = h_hi = None   # produced by layer 0's epilogue

            for layer in range(N_LAYERS):
                wlo, whi = Ws[layer]
                if layer == 0:
                    # layer 1 aggregates the host-provided pre-scaled x table;
                    # W1 is applied AFTER aggregation (linearity) — no GEMM,
                    # no AllGather.
                    tbl_lo_ap = x_tbl_in[0:min(LO_LIMIT, n_nodes), :]
                    tbl_hi_ap = (x_tbl_in[LO_LIMIT:n_nodes, :]
                                 if NHI_ROWS else x_tbl_in[0:1, :])
                else:
                    # GEMM blocks were emitted inline at the end of the previous
                    # layer's per-block loop; only the AllGather remains here.
                    # ---- AllGather the table ----
                    nc.gpsimd.collective_compute(
                        "AllGather", mybir.AluOpType.bypass,
                        replica_groups=[list(range(NCORES))],
                        ins=[y_loc[:, :]], outs=[y_full[:, :]],
                    )
                    tbl_lo_ap = y_full[0:min(LO_LIMIT, n_nodes), :]
                    tbl_hi_ap = (y_full[LO_LIMIT:n_nodes, :]
                                 if NHI_ROWS else y_full[0:1, :])

                # ---- gathers ----
                def emit_gathers(calls, gpool, gidx, table_ap, tagn, qn=0):
                    tiles = []
                    for (c0, cnt) in calls:
                        gt = gpool.tile([P, G_CALL * hid],
                                        dt.bfloat16 if layer == 0 else tbl_dt, tag=tagn)
                        g = nc.gpsimd.dma_gather(
                            out_ap=gt[:, :cnt * hid].rearrange("p (g f) -> p g f", g=cnt),
                            in_ap=table_ap,
                            idxs_ap=gidx[:, c0 * 8:(c0 + cnt) * 8],
                            num_idxs=cnt * P,
                            num_idxs_reg=nidx_reg(cnt * P),
                            elem_size=hid,
                            queue_num=qn % N_QUEUES,
                        )
                        add_dep_helper(g.ins, lib_inst.ins, sync=False, reason="lib first")
                        tiles.append(gt)
                    return tiles

                # emit lo/hi gather calls interleaved in block-consumption order:
                # a block's PSUM accumulation needs BOTH its lo and hi chunks, so
                # emitting all lo calls first would stall every block on the
                # first hi call (~100us into the phase).
                def emit_gathers_merged():
                    tiles_lo = [None] * len(calls_lo)
                    tiles_hi = [None] * len(calls_hi)
                    def emit_one(kind, i):
                        if kind == 0:
                            tiles_lo[i] = emit_gathers([calls_lo[i]], glo_pool,
                                                       gidx_lo, tbl_lo_ap, "glo")[0]
                        else:
                            tiles_hi[i] = emit_gathers([calls_hi[i]], ghi_pool,
                                                       gidx_hi, tbl_hi_ap, "ghi", qn=1)[0]
                    def last_blk(starts, c0, cnt):
                        return int(np.searchsorted(starts, c0 + cnt - 1, side="right") - 1)
                    order = ([(last_blk(lo_start, c0, cnt), 0, i)
                              for i, (c0, cnt) in enumerate(calls_lo)] +
                             [(last_blk(hi_start, c0, cnt), 1, i)
                              for i, (c0, cnt) in enumerate(calls_hi)])
                    for _, kind, i in sorted(order):
                        emit_one(kind, i)
                    return tiles_lo, tiles_hi

                gt_lo, gt_hi = emit_gathers_merged()

                # ---- batched one-hot compares ----
                def emit_compares(batches, dstid, tagn, expand=False):
                    tiles = []
                    for (c0, cnt) in batches:
                        st = spool.tile([P, KB * P], dt.bfloat16, tag=tagn)
                        if expand:
                            # materialize the dst-id broadcast on ACT so the DVE
                            # is_equal sees packed last dims (2x DVE mode)
                            tmp = xpool.tile([P, KB * P], dt.bfloat16, tag="dx")
                            nc.scalar.activation(
                                tmp[:, :cnt * P].rearrange("p (c d) -> p c d", c=cnt),
                                dstid[:, c0:c0 + cnt, None].to_broadcast([P, cnt, P]),
                                mybir.ActivationFunctionType.Copy, scale=1.0)
                            in0 = tmp[:, :cnt * P].rearrange("p (c d) -> p c d", c=cnt)
                        else:
                            in0 = dstid[:, c0:c0 + cnt, None].to_broadcast([P, cnt, P])
                        nc.vector.tensor_tensor(
                            out=st[:, :cnt * P].rearrange("p (c d) -> p c d", c=cnt),
                            in0=in0,
                            in1=iota[:, None, :].to_broadcast([P, cnt, P]),
                            op=mybir.AluOpType.is_equal,
                        )
                        tiles.append(st)
                    return tiles

                # compares likewise interleaved by consuming block
                def emit_compares_merged():
                    tiles_lo = [None] * len(batches_lo)
                    tiles_hi = [None] * len(batches_hi)
                    def last_blk(starts, c0, cnt):
                        return int(np.searchsorted(starts, c0 + cnt - 1, side="right") - 1)
                    order = ([(last_blk(lo_start, c0, cnt), 0, i)
                              for i, (c0, cnt) in enumerate(batches_lo)] +
                             [(last_blk(hi_start, c0, cnt), 1, i)
                              for i, (c0, cnt) in enumerate(batches_hi)])
                    for n_emit, (_, kind, i) in enumerate(sorted(order)):
                        xp = (XP_MOD > 0) and (n_emit % XP_MOD != 0)
                        if kind == 0:
                            tiles_lo[i] = emit_compares([batches_lo[i]], dstid_lo, "stlo", xp)[0]
                        else:
                            tiles_hi[i] = emit_compares([batches_hi[i]], dstid_hi, "sthi", xp)[0]
                    return tiles_lo, tiles_hi

                st_lo, st_hi = emit_compares_merged()

                # ---- per-block accumulate + epilogue ----
                h_lo_new = hpool.tile([P, n_loc], dt.bfloat16, tag="h0")
                h_hi_new = hpool.tile([P, n_loc], dt.bfloat16, tag="h1")

                def emit_gemm_block(nlayer, gb, hlo_t, hhi_t):
                    # next layer's GEMM for one block: y = s .* (h @ W) -> y_loc
                    gbs = gb * P
                    gnb = min(P, n_loc - gbs)
                    wnlo, wnhi = Ws[nlayer]
                    gps = psy.tile([P, hid], dt.float32, tag="ypsum")
                    nc.tensor.matmul(out=gps[:gnb, :], lhsT=hlo_t[:, gbs:gbs + gnb],
                                     rhs=wnlo[:, :], start=True, stop=False)
                    nc.tensor.matmul(out=gps[:gnb, :], lhsT=hhi_t[:, gbs:gbs + gnb],
                                     rhs=wnhi[:, :], start=False, stop=True)
                    gysb = wpool.tile([P, hid], tbl_dt, tag="ysb")
                    nc.scalar.activation(gysb[:gnb, :], gps[:gnb, :],
                                         mybir.ActivationFunctionType.Copy,
                                         scale=s_col[:gnb, gb:gb + 1])
                    nc.sync.dma_start(out=y_loc[gbs:gbs + gnb, :], in_=gysb[:gnb, :])
                for b in range(n_blk):
                    bs = b * P
                    nb = min(P, n_loc - bs)
                    seq = [(gt_lo, st_lo, c) for c in range(lo_start[b], lo_start[b + 1])] + \
                          [(gt_hi, st_hi, c) for c in range(hi_start[b], hi_start[b + 1])]

                    def scatter_psum(h):
                        ps = pst.tile([P, P], dt.float32, tag="tpsum")
                        for i, (gts, sts, c) in enumerate(seq):
                            gt = gts[c // G_CALL]
                            goff = (c % G_CALL) * hid + h * P
                            st = sts[c // KB]
                            soff = (c % KB) * P
                            nc.tensor.matmul(
                                out=ps[:, :], lhsT=gt[:, goff:goff + P],
                                rhs=st[:, soff:soff + P],
                                start=(i == 0), stop=(i == len(seq) - 1),
                            )
                        return ps

                    def epilogue(ps, h, h_new):
                        tmp = epool.tile([P, P], dt.float32, tag="eptmp")
                        nc.vector.tensor_tensor(out=tmp[:, :nb], in0=ps[:, :nb],
                                                in1=s_bc[:, bs:bs + nb],
                                                op=mybir.AluOpType.mult)
                        a_ap = aff[:, 4 * layer + h:4 * layer + h + 1]
                        cc_ap = aff[:, 4 * layer + 2 + h:4 * layer + 3 + h]
                        nc.scalar.activation(h_new[:, bs:bs + nb], tmp[:, :nb],
                                             mybir.ActivationFunctionType.Relu,
                                             bias=cc_ap, scale=a_ap)

                    if layer == 0:
                        # aggregate raw x, then apply W1 per block
                        traws = []
                        for h in (0, 1):
                            ps = scatter_psum(h)
                            tr = wpool.tile([P, P], dt.bfloat16, tag=f"traw{h}")
                            nc.scalar.activation(tr[:, :nb], ps[:, :nb],
                                                 mybir.ActivationFunctionType.Copy,
                                                 scale=1.0)
                            traws.append(tr)
                        for h, h_new in ((0, h_lo_new), (1, h_hi_new)):
                            pw = ptw.tile([P, P], dt.float32, tag="twps")
                            nc.tensor.matmul(out=pw[:, :nb],
                                             lhsT=wlo[:, h * P:(h + 1) * P],
                                             rhs=traws[0][:, :nb],
                                             start=True, stop=False)
                            nc.tensor.matmul(out=pw[:, :nb],
                                             lhsT=whi[:, h * P:(h + 1) * P],
                                             rhs=traws[1][:, :nb],
                                             start=False, stop=True)
                            epilogue(pw, h, h_new)
                    else:
                        for h, h_new in ((0, h_lo_new), (1, h_hi_new)):
                            ps = scatter_psum(h)
                            epilogue(ps, h, h_new)

                    if layer < N_LAYERS - 1 and b >= GEMM_DLY:
                        emit_gemm_block(layer + 1, b - GEMM_DLY, h_lo_new, h_hi_new)
                if layer < N_LAYERS - 1:
                    for b in range(max(0, n_blk - GEMM_DLY), n_blk):
                        emit_gemm_block(layer + 1, b, h_lo_new, h_hi_new)
                h_lo, h_hi = h_lo_new, h_hi_new

            # ---- FC + sigmoid ----
            osb = cpool.tile([1, n_loc], dt.float32)
            for t0 in range(0, n_loc, 512):
                w = min(512, n_loc - t0)
                ps = psf.tile([1, 512], dt.float32, tag="fcps")
                nc.tensor.matmul(out=ps[:1, :w], lhsT=wfc[:, 0:1],
                                 rhs=h_lo[:, t0:t0 + w], start=True, stop=False)
                nc.tensor.matmul(out=ps[:1, :w], lhsT=wfc[:, 1:2],
                                 rhs=h_hi[:, t0:t0 + w], start=False, stop=True)
                nc.scalar.activation(osb[:1, t0:t0 + w], ps[:1, :w],
                                     mybir.ActivationFunctionType.Sigmoid,
                                     bias=float(bfc_val), scale=1.0)
            nc.sync.dma_start(out=out_ext[:, :], in_=osb[:])

    mybir.codegen_inst_isa_subclasses(nc)
    _split_multiwaits(nc)
    return nc


def kernel(**inputs):
    global LAST_RESULTS, LAST_NC
    x = np.asarray(inputs["x"], dtype=np.float32)
    edge_index = np.asarray(inputs["edge_index"])
    n_nodes, feat = x.shape
    hid = np.asarray(inputs["W1"]).shape[1]

    cores, nlo, nhi, NCL, NCH, n_loc, n_blk, s_all = _prep_host(x, edge_index, n_nodes)
    x_tbl = (s_all[:, None] * x).astype(BF16)   # pre-scaled input table, same on all cores

    # BN affine folding: z = (agg + b - m) * a + be,  a = g * rsqrt(v + eps)
    aff = np.zeros((P, 12), np.float32)
    for i in (1, 2, 3):
        g = np.asarray(inputs[f"g{i}"], np.float32)
        be = np.asarray(inputs[f"be{i}"], np.float32)
        m = np.asarray(inputs[f"m{i}"], np.float32)
        v = np.asarray(inputs[f"v{i}"], np.float32)
        b = np.asarray(inputs[f"b{i}"], np.float32)
        a = g / np.sqrt(v + BN_EPS)
        cc = (b - m) * a + be
        L = i - 1
        aff[:, 4 * L + 0] = a[0:P]
        aff[:, 4 * L + 1] = a[P:2 * P]
        aff[:, 4 * L + 2] = cc[0:P]
        aff[:, 4 * L + 3] = cc[P:2 * P]

    wfc_np = np.zeros((P, 2), dtype=BF16)
    Wfc = np.asarray(inputs["Wfc"], np.float32)
    wfc_np[:, 0] = Wfc[0:P, 0].astype(BF16)
    wfc_np[:, 1] = Wfc[P:2 * P, 0].astype(BF16)
    bfc_val = float(np.asarray(inputs["bfc"]).reshape(-1)[0])
    iota_np = np.tile(np.arange(P, dtype=np.float32).astype(BF16)[None, :], (P, 1))

    nc = _build(n_nodes, n_loc, n_blk, nlo, nhi, NCL, NCH, feat, hid, bfc_val)

    in_maps = []
    for c in range(NCORES):
        d = cores[c]
        in_maps.append({
            "x_tbl": x_tbl,
            "W1": np.asarray(inputs["W1"], np.float32).astype(BF16),
            "W2": np.asarray(inputs["W2"], np.float32).astype(BF16),
            "W3": np.asarray(inputs["W3"], np.float32).astype(BF16),
            "wfc": wfc_np, "aff": aff,
            "s_col": d["s_col"], "s_bcast": d["s_bcast"], "iota": iota_np,
            "gidx_lo": d["gidx_lo"], "gidx_hi": d["gidx_hi"],
            "dstid_lo": d["dstid_lo"], "dstid_hi": d["dstid_hi"],
        })

    res = run_bass_kernel_spmd(nc, in_maps, core_ids=list(range(NCORES)))
    LAST_RESULTS = res
    globals()["LAST_NC"] = nc
    out = np.concatenate([res.results[c]["out"].reshape(-1) for c in range(NCORES)])
    return out.reshape(-1, 1).astype(np.float32)



# revision 47
# speedup vs baseline: 1.6318x; 1.0000x over previous
"""GCN (3x GCNConv+BN+ReLU, FC+sigmoid) on 8 Trainium2 NeuronCores.

Strategy (node-sharded, graph structure preprocessed on host):
  - Nodes sharded 8-ways (6250/core). Edges partitioned by destination core,
    sorted by destination block (128 dsts), padded to 128-edge chunks.
  - Layer 1 exploits linearity (agg(x@W1) == agg(x)@W1): the host ships the
    full pre-scaled input table x_tbl = s .* x (bf16, identical per core), so
    layer 1 has NO GEMM and NO AllGather — it aggregates raw x rows and
    applies W1 per destination block afterwards (PSUM -> bf16 copy, then
    2x2 half matmuls), saving one full-table collective.
  - Layers 2-3: GEMM y = s .* (h @ W) per-core is emitted inline (delayed by
    GEMM_DLY blocks to avoid PE head-of-line stalls) inside the previous
    layer's block loop; the fp8e4 node table (halves the collective's modeled
    bytes; rel err stays ~2e-3) is AllGathered, then dma_gather (SWDGE) pulls
    y[src] per edge chunk and a one-hot S matrix (DVE is_equal against an
    iota row) turns segment-sum into PSUM-accumulated matmuls:
    t[f,d] += msg.T @ S. Epilogue: h' = relu((t * s_dst) * a_f + cc_f) with
    BN folded into a/cc, written feature-major as the next layer's lhsT.
  - Gather-table indices are int16 (SWDGE limit), so the node table is split
    at row 32768 into "lo"/"hi" streams. Gather calls and compare batches for
    the two streams are emitted interleaved in block-consumption order — each
    block's PSUM accumulation needs both streams, so emitting all lo calls
    first would stall every block on the first hi gather (~100us).
  - Final FC + sigmoid on PE/ACT; output assembled on host.
"""
import os
import sys
sys.path.insert(0, "/opt/trn_rl_repo")

import numpy as np
import ml_dtypes

import concourse.bass as bass
import concourse.tile as tile
from concourse import mybir
from concourse.bass_utils import run_bass_kernel_spmd
from concourse.library_config import mlp as LIB_MLP
from concourse.tile_rust import add_dep_helper

BF16 = ml_dtypes.bfloat16
P = 128
NCORES = 8
BN_EPS = 1e-5
LO_LIMIT = 32768       # int16 index limit for the gather table
PAD_DST = 200.0        # out-of-range dst id for padding edges

LAST_RESULTS = None    # test harness reads exec_time from here
LAST_NC = None         # built program, for cost-model timing in test.py

N_LAYERS = 3

# sweep knobs (env; defaults = shipped config)
GLO_BUFS = int(os.environ.get("SW_GLO", "5"))
GHI_BUFS = int(os.environ.get("SW_GHI", "4"))
ST_BUFS = int(os.environ.get("SW_STB", "5"))
FP8_TBL = int(os.environ.get("SW_FP8", "1"))
XP_MOD = int(os.environ.get("SW_XP", "2"))  # expand dstid via ACT for batches with idx % XP_MOD != 0
TPS_BUFS = int(os.environ.get("SW_TPS", "3"))
TW_BUFS = int(os.environ.get("SW_TW", "2"))
YSB_BUFS = int(os.environ.get("SW_YSB", "3"))
N_QUEUES = int(os.environ.get("SW_NQ", "1"))
KB = int(os.environ.get("SW_KB", "16"))
GEMM_DLY = int(os.environ.get("SW_GD", "9"))
G_CALL = int(os.environ.get("SW_GC", "8"))  # chunks (of 128 edges) per dma_gather call; 8*128 descs fit the 16KB SWDGE ring


def _split_multiwaits(nc):
    """This walrus build allows one sync-wait per instruction; move extras
    onto preceding same-engine NoOps."""
    n_new = 0
    for fn in nc.m.functions:
        for blk in fn.blocks:
            out = []
            changed = False
            for ins in list(blk.instructions):
                si = ins.sync_info
                if si is not None and len(si.on_wait) > 1:
                    waits = list(si.on_wait)
                    for w in waits[:-1]:
                        n_new += 1
                        out.append(mybir.InstNoOp(
                            name=f"I-mwsplit-{n_new}", engine=ins.engine,
                            sync_info=mybir.SyncInfo(on_wait=[w], on_update=[])))
                    si.on_wait = [waits[-1]]
                    changed = True
                out.append(ins)
            if changed:
                blk.instructions = out


def _prep_host(x, edge_index, n_nodes):
    """Shard + sort + pad the graph. Returns per-core tensors and the common
    (cross-core) block/chunk structure."""
    n_loc = n_nodes // NCORES
    n_blk = (n_loc + P - 1) // P

    src = np.concatenate([edge_index[0], np.arange(n_nodes, dtype=np.int64)])
    dst = np.concatenate([edge_index[1], np.arange(n_nodes, dtype=np.int64)])
    deg = np.bincount(dst, minlength=n_nodes).astype(np.float32)
    s = (1.0 / np.sqrt(np.maximum(deg, 1.0))).astype(np.float32)

    # per-core, per-block edge lists split by src range
    per_core = []
    for c in range(NCORES):
        mask = (dst >= c * n_loc) & (dst < (c + 1) * n_loc)
        cs, cd = src[mask], dst[mask] - c * n_loc
        blk = cd // P
        order = np.argsort(blk, kind="stable")
        cs, cd, blk = cs[order], cd[order], blk[order]
        lo_lists, hi_lists = [], []
        for b in range(n_blk):
            m = blk == b
            bs, bd = cs[m], cd[m] - b * P
            lo = bs < LO_LIMIT
            lo_lists.append((bs[lo], bd[lo]))
            hi_lists.append((bs[~lo] - LO_LIMIT, bd[~lo]))
        per_core.append((lo_lists, hi_lists))

    # common per-block chunk counts = max over cores (>=1 lo chunk per block)
    nlo = np.zeros(n_blk, np.int64)
    nhi = np.zeros(n_blk, np.int64)
    for c in range(NCORES):
        lo_lists, hi_lists = per_core[c]
        for b in range(n_blk):
            nlo[b] = max(nlo[b], (len(lo_lists[b][0]) + P - 1) // P)
            nhi[b] = max(nhi[b], (len(hi_lists[b][0]) + P - 1) // P)
    nlo = np.maximum(nlo, 1)
    NCL, NCH = int(nlo.sum()), int(nhi.sum())

    def pack(lists, n_chunks_per_blk, total_chunks):
        """Build gidx [128, total*8] int16 (16-wrap, x8 replicated) and
        dstid [128, total] bf16 for one stream."""
        gsrc = np.zeros(total_chunks * P, np.int64)
        gdst = np.full(total_chunks * P, PAD_DST, np.float32)
        pos = 0
        for b in range(len(n_chunks_per_blk)):
            bs, bd = lists[b]
            n = len(bs)
            cap = int(n_chunks_per_blk[b]) * P
            gsrc[pos:pos + n] = bs
            gdst[pos:pos + n] = bd
            pos += cap
        j = np.arange(total_chunks * P)
        gidx16 = np.zeros((16, total_chunks * 8), np.int16)
        gidx16[j % 16, j // 16] = gsrc
        gidx = np.tile(gidx16, (8, 1))
        dstid = np.zeros((P, total_chunks), dtype=BF16)
        dstid[j % P, j // P] = gdst.astype(BF16)
        return gidx, dstid

    cores = []
    for c in range(NCORES):
        lo_lists, hi_lists = per_core[c]
        gidx_lo, dstid_lo = pack(lo_lists, nlo, NCL)
        gidx_hi, dstid_hi = pack(hi_lists, nhi, NCH)

        s_loc = s[c * n_loc:(c + 1) * n_loc]
        s_col = np.zeros((P, n_blk), np.float32)
        for b in range(n_blk):
            nb = min(P, n_loc - b * P)
            s_col[:nb, b] = s_loc[b * P:b * P + nb]
        s_bcast = np.tile(s_loc[None, :], (P, 1)).astype(BF16)

        cores.append(dict(gidx_lo=gidx_lo, gidx_hi=gidx_hi,
                          dstid_lo=dstid_lo, dstid_hi=dstid_hi,
                          s_col=s_col, s_bcast=s_bcast))
    return cores, nlo, nhi, NCL, NCH, n_loc, n_blk, s


def _build(n_nodes, n_loc, n_blk, nlo, nhi, NCL, NCH, feat, hid, bfc_val):
    nc = bass.Bass(num_swdge_queues=N_QUEUES,
                   dynamic_dma_scratch_size=max(16384, G_CALL * P * 16))
    dt = mybir.dt
    NHI_ROWS = n_nodes - LO_LIMIT if n_nodes > LO_LIMIT else 0

    # full pre-scaled input table: x_tbl[n] = s_n * x[n]  (layer-1 aggregates this
    # directly — agg(x@W1) == agg(x)@W1 — so no GEMM/AllGather before layer 1)
    x_tbl_in = nc.declare_dram_parameter("x_tbl", [n_nodes, feat], dt.bfloat16, isOutput=False)
    W_in = [nc.declare_dram_parameter(f"W{i}", [feat if i == 1 else hid, hid], dt.bfloat16, isOutput=False)
            for i in (1, 2, 3)]
    wfc_in = nc.declare_dram_parameter("wfc", [P, 2], dt.bfloat16, isOutput=False)
    aff_in = nc.declare_dram_parameter("aff", [P, 12], dt.float32, isOutput=False)
    s_col_in = nc.declare_dram_parameter("s_col", [P, n_blk], dt.float32, isOutput=False)
    s_bc_in = nc.declare_dram_parameter("s_bcast", [P, n_loc], dt.bfloat16, isOutput=False)
    iota_in = nc.declare_dram_parameter("iota", [P, P], dt.bfloat16, isOutput=False)
    gidx_lo_in = nc.declare_dram_parameter("gidx_lo", [P, NCL * 8], dt.int16, isOutput=False)
    gidx_hi_in = nc.declare_dram_parameter("gidx_hi", [P, NCH * 8], dt.int16, isOutput=False)
    dstid_lo_in = nc.declare_dram_parameter("dstid_lo", [P, NCL], dt.bfloat16, isOutput=False)
    dstid_hi_in = nc.declare_dram_parameter("dstid_hi", [P, NCH], dt.bfloat16, isOutput=False)
    out_ext = nc.declare_dram_parameter("out", [1, n_loc], dt.float32, isOutput=True)

    tbl_dt = dt.float8e4 if FP8_TBL else dt.bfloat16
    y_loc = nc.dram_tensor("y_loc", [n_loc, hid], tbl_dt)
    y_full = nc.dram_tensor("y_full", [n_nodes, hid], tbl_dt, addr_space="Shared")

    lo_start = np.concatenate([[0], np.cumsum(nlo)])
    hi_start = np.concatenate([[0], np.cumsum(nhi)])
    calls_lo = [(c0, min(G_CALL, NCL - c0)) for c0 in range(0, NCL, G_CALL)]
    calls_hi = [(c0, min(G_CALL, NCH - c0)) for c0 in range(0, NCH, G_CALL)]
    batches_lo = [(c0, min(KB, NCL - c0)) for c0 in range(0, NCL, KB)]
    batches_hi = [(c0, min(KB, NCH - c0)) for c0 in range(0, NCH, KB)]

    with tile.TileContext(nc) as tc:
        with tc.tile_pool(name="const", bufs=1) as cpool, \
             tc.tile_pool(name="ht", bufs=2) as hpool, \
             tc.tile_pool(name="glo", bufs=GLO_BUFS) as glo_pool, \
             tc.tile_pool(name="ghi", bufs=GHI_BUFS) as ghi_pool, \
             tc.tile_pool(name="work", bufs=YSB_BUFS) as wpool, \
             tc.tile_pool(name="ep", bufs=2) as epool, \
             tc.tile_pool(name="stp", bufs=ST_BUFS) as spool, \
             tc.tile_pool(name="dstx", bufs=3) as xpool, \
             tc.tile_pool(name="psy", bufs=2, space="PSUM") as psy, \
             tc.tile_pool(name="pst", bufs=TPS_BUFS, space="PSUM") as pst, \
             tc.tile_pool(name="ptw", bufs=TW_BUFS, space="PSUM") as ptw, \
             tc.tile_pool(name="psf", bufs=1, space="PSUM") as psf:

            lib_inst = nc.gpsimd.load_library(LIB_MLP)

            # to_reg leaks a Pool register per call; cache per distinct count
            _nreg = {}

            def nidx_reg(n):
                if n not in _nreg:
                    _nreg[n] = nc.gpsimd.to_reg(n)
                return _nreg[n]

            # ---- constants ----
            iota = cpool.tile([P, P], dt.bfloat16)
            nc.sync.dma_start(out=iota[:], in_=iota_in[:, :])
            gidx_lo = cpool.tile([P, NCL * 8], dt.int16)
            nc.sync.dma_start(out=gidx_lo[:], in_=gidx_lo_in[:, :])
            gidx_hi = cpool.tile([P, NCH * 8], dt.int16)
            nc.sync.dma_start(out=gidx_hi[:], in_=gidx_hi_in[:, :])
            dstid_lo = cpool.tile([P, NCL], dt.bfloat16)
            nc.sync.dma_start(out=dstid_lo[:], in_=dstid_lo_in[:, :])
            dstid_hi = cpool.tile([P, NCH], dt.bfloat16)
            nc.sync.dma_start(out=dstid_hi[:], in_=dstid_hi_in[:, :])
            s_col = cpool.tile([P, n_blk], dt.float32)
            nc.sync.dma_start(out=s_col[:], in_=s_col_in[:, :])
            s_bc = cpool.tile([P, n_loc], dt.bfloat16)
            nc.sync.dma_start(out=s_bc[:], in_=s_bc_in[:, :])
            aff = cpool.tile([P, 12], dt.float32)
            nc.sync.dma_start(out=aff[:], in_=aff_in[:, :])
            wfc = cpool.tile([P, 2], dt.bfloat16)
            nc.sync.dma_start(out=wfc[:], in_=wfc_in[:, :])
            Ws = []
            for i in range(3):
                wlo = cpool.tile([P, hid], dt.bfloat16, tag=f"w{i}lo")
                nc.sync.dma_start(out=wlo[:], in_=W_in[i][0:P, :])
                whi = cpool.tile([P, hid], dt.bfloat16, tag=f"w{i}hi")
                nc.sync.dma_start(out=whi[:], in_=W_in[i][P:2 * P, :])
                Ws.append((wlo, whi))

            h_lo = h_hi = None   # produced by layer 0's epilogue

            for layer in range(N_LAYERS):
                wlo, whi = Ws[layer]
                if layer == 0:
                    # layer 1 aggregates the host-provided pre-scaled x table;
                    # W1 is applied AFTER aggregation (linearity) — no GEMM,
                    # no AllGather.
                    tbl_lo_ap = x_tbl_in[0:min(LO_LIMIT, n_nodes), :]
                    tbl_hi_ap = (x_tbl_in[LO_LIMIT:n_nodes, :]
                                 if NHI_ROWS else x_tbl_in[0:1, :])
                else:
                    # GEMM blocks were emitted inline at the end of the previous
                    # layer's per-block loop; only the AllGather remains here.
                    # ---- AllGather the table ----
                    nc.gpsimd.collective_compute(
                        "AllGather", mybir.AluOpType.bypass,
                        replica_groups=[list(range(NCORES))],
                        ins=[y_loc[:, :]], outs=[y_full[:, :]],
                    )
                    tbl_lo_ap = y_full[0:min(LO_LIMIT, n_nodes), :]
                    tbl_hi_ap = (y_full[LO_LIMIT:n_nodes, :]
                                 if NHI_ROWS else y_full[0:1, :])

                # ---- gathers ----
                def emit_gathers(calls, gpool, gidx, table_ap, tagn, qn=0):
                    tiles = []
                    for (c0, cnt) in calls:
                        gt = gpool.tile([P, G_CALL * hid],
                                        dt.bfloat16 if layer == 0 else tbl_dt, tag=tagn)
                        g = nc.gpsimd.dma_gather(
                            out_ap=gt[:, :cnt * hid].rearrange("p (g f) -> p g f", g=cnt),
                            in_ap=table_ap,
                            idxs_ap=gidx[:, c0 * 8:(c0 + cnt) * 8],
                            num_idxs=cnt * P,
                            num_idxs_reg=nidx_reg(cnt * P),
                            elem_size=hid,
                            queue_num=qn % N_QUEUES,
                        )
                        add_dep_helper(g.ins, lib_inst.ins, sync=False, reason="lib first")
                        tiles.append(gt)
                    return tiles

                # emit lo/hi gather calls interleaved in block-consumption order:
                # a block's PSUM accumulation needs BOTH its lo and hi chunks, so
                # emitting all lo calls first would stall every block on the
                # first hi call (~100us into the phase).
                def emit_gathers_merged():
                    tiles_lo = [None] * len(calls_lo)
                    tiles_hi = [None] * len(calls_hi)
                    def emit_one(kind, i):
                        if kind == 0:
                            tiles_lo[i] = emit_gathers([calls_lo[i]], glo_pool,
                                                       gidx_lo, tbl_lo_ap, "glo")[0]
                        else:
                            tiles_hi[i] = emit_gathers([calls_hi[i]], ghi_pool,
                                                       gidx_hi, tbl_hi_ap, "ghi", qn=1)[0]
                    def last_blk(starts, c0, cnt):
                        return int(np.searchsorted(starts, c0 + cnt - 1, side="right") - 1)
                    order = ([(last_blk(lo_start, c0, cnt), 0, i)
                              for i, (c0, cnt) in enumerate(calls_lo)] +
                             [(last_blk(hi_start, c0, cnt), 1, i)
                              for i, (c0, cnt) in enumerate(calls_hi)])
                    for _, kind, i in sorted(order):
                        emit_one(kind, i)
                    return tiles_lo, tiles_hi

                gt_lo, gt_hi = emit_gathers_merged()

                # ---- batched one-hot compares ----
                def emit_compares(batches, dstid, tagn, expand=False):
                    tiles = []
                    for (c0, cnt) in batches:
                        st = spool.tile([P, KB * P], dt.bfloat16, tag=tagn)
                        if expand:
                            # materialize the dst-id broadcast on ACT so the DVE
                            # is_equal sees packed last dims (2x DVE mode)
                            tmp = xpool.tile([P, KB * P], dt.bfloat16, tag="dx")
                            nc.scalar.activation(
                                tmp[:, :cnt * P].rearrange("p (c d) -> p c d", c=cnt),
                                dstid[:, c0:c0 + cnt, None].to_broadcast([P, cnt, P]),
                                mybir.ActivationFunctionType.Copy, scale=1.0)
                            in0 = tmp[:, :cnt * P].rearrange("p (c d) -> p c d", c=cnt)
                        else:
                            in0 = dstid[:, c0:c0 + cnt, None].to_broadcast([P, cnt, P])
                        nc.vector.tensor_tensor(
                            out=st[:, :cnt * P].rearrange("p (c d) -> p c d", c=cnt),
                            in0=in0,
                            in1=iota[:, None, :].to_broadcast([P, cnt, P]),
                            op=mybir.AluOpType.is_equal,
                        )
                        tiles.append(st)
                    return tiles

                # compares likewise interleaved by consuming block
                def emit_compares_merged():
                    tiles_lo = [None] * len(batches_lo)
                    tiles_hi = [None] * len(batches_hi)
                    def last_blk(starts, c0, cnt):
                        return int(np.searchsorted(starts, c0 + cnt - 1, side="right") - 1)
                    order = ([(last_blk(lo_start, c0, cnt), 0, i)
                              for i, (c0, cnt) in enumerate(batches_lo)] +
                             [(last_blk(hi_start, c0, cnt), 1, i)
                              for i, (c0, cnt) in enumerate(batches_hi)])
                    for n_emit, (_, kind, i) in enumerate(sorted(order)):
                        xp = (XP_MOD > 0) and (n_emit % XP_MOD != 0)
                        if kind == 0:
                            tiles_lo[i] = emit_compares([batches_lo[i]], dstid_lo, "stlo", xp)[0]
                        else:
                            tiles_hi[i] = emit_compares([batches_hi[i]], dstid_hi, "sthi", xp)[0]
                    return tiles_lo, tiles_hi

                st_lo, st_hi = emit_compares_merged()

                # ---- per-block accumulate + epilogue ----
                h_lo_new = hpool.tile([P, n_loc], dt.bfloat16, tag="h0")
                h_hi_new = hpool.tile([P, n_loc], dt.bfloat16, tag="h1")

                if layer == N_LAYERS - 1:
                    osb = cpool.tile([1, n_loc], dt.float32)
                    fc_done = set()

                    def emit_fc_chunk(k):
                        # FC+sigmoid for columns [512k, 512k+512): needs blocks
                        # 4k..4k+3 of the final h to be complete
                        if k in fc_done:
                            return
                        fc_done.add(k)
                        t0 = k * 512
                        w = min(512, n_loc - t0)
                        fps = psf.tile([1, 512], dt.float32, tag="fcps")
                        nc.tensor.matmul(out=fps[:1, :w], lhsT=wfc[:, 0:1],
                                         rhs=h_lo_new[:, t0:t0 + w], start=True, stop=False)
                        nc.tensor.matmul(out=fps[:1, :w], lhsT=wfc[:, 1:2],
                                         rhs=h_hi_new[:, t0:t0 + w], start=False, stop=True)
                        nc.scalar.activation(osb[:1, t0:t0 + w], fps[:1, :w],
                                             mybir.ActivationFunctionType.Sigmoid,
                                             bias=float(bfc_val), scale=1.0)

                def emit_gemm_block(nlayer, gb, hlo_t, hhi_t):
                    # next layer's GEMM for one block: y = s .* (h @ W) -> y_loc
                    gbs = gb * P
                    gnb = min(P, n_loc - gbs)
                    wnlo, wnhi = Ws[nlayer]
                    gps = psy.tile([P, hid], dt.float32, tag="ypsum")
                    nc.tensor.matmul(out=gps[:gnb, :], lhsT=hlo_t[:, gbs:gbs + gnb],
                                     rhs=wnlo[:, :], start=True, stop=False)
                    nc.tensor.matmul(out=gps[:gnb, :], lhsT=hhi_t[:, gbs:gbs + gnb],
                                     rhs=wnhi[:, :], start=False, stop=True)
                    gysb = wpool.tile([P, hid], tbl_dt, tag="ysb")
                    nc.scalar.activation(gysb[:gnb, :], gps[:gnb, :],
                                         mybir.ActivationFunctionType.Copy,
                                         scale=s_col[:gnb, gb:gb + 1])
                    nc.sync.dma_start(out=y_loc[gbs:gbs + gnb, :], in_=gysb[:gnb, :])
                for b in range(n_blk):
                    bs = b * P
                    nb = min(P, n_loc - bs)
                    seq = [(gt_lo, st_lo, c) for c in range(lo_start[b], lo_start[b + 1])] + \
                          [(gt_hi, st_hi, c) for c in range(hi_start[b], hi_start[b + 1])]

                    def scatter_psum(h):
                        ps = pst.tile([P, P], dt.float32, tag="tpsum")
                        for i, (gts, sts, c) in enumerate(seq):
                            gt = gts[c // G_CALL]
                            goff = (c % G_CALL) * hid + h * P
                            st = sts[c // KB]
                            soff = (c % KB) * P
                            nc.tensor.matmul(
                                out=ps[:, :], lhsT=gt[:, goff:goff + P],
                                rhs=st[:, soff:soff + P],
                                start=(i == 0), stop=(i == len(seq) - 1),
                            )
                        return ps

                    def epilogue(ps, h, h_new):
                        tmp = epool.tile([P, P], dt.float32, tag="eptmp")
                        nc.vector.tensor_tensor(out=tmp[:, :nb], in0=ps[:, :nb],
                                                in1=s_bc[:, bs:bs + nb],
                                                op=mybir.AluOpType.mult)
                        a_ap = aff[:, 4 * layer + h:4 * layer + h + 1]
                        cc_ap = aff[:, 4 * layer + 2 + h:4 * layer + 3 + h]
                        nc.scalar.activation(h_new[:, bs:bs + nb], tmp[:, :nb],
                                             mybir.ActivationFunctionType.Relu,
                                             bias=cc_ap, scale=a_ap)

                    if layer == 0:
                        # aggregate raw x, then apply W1 per block
                        traws = []
                        for h in (0, 1):
                            ps = scatter_psum(h)
                            tr = wpool.tile([P, P], dt.bfloat16, tag=f"traw{h}")
                            nc.scalar.activation(tr[:, :nb], ps[:, :nb],
                                                 mybir.ActivationFunctionType.Copy,
                                                 scale=1.0)
                            traws.append(tr)
                        for h, h_new in ((0, h_lo_new), (1, h_hi_new)):
                            pw = ptw.tile([P, P], dt.float32, tag="twps")
                            nc.tensor.matmul(out=pw[:, :nb],
                                             lhsT=wlo[:, h * P:(h + 1) * P],
                                             rhs=traws[0][:, :nb],
                                             start=True, stop=False)
                            nc.tensor.matmul(out=pw[:, :nb],
                                             lhsT=whi[:, h * P:(h + 1) * P],
                                             rhs=traws[1][:, :nb],
                                             start=False, stop=True)
                            epilogue(pw, h, h_new)
                    else:
                        for h, h_new in ((0, h_lo_new), (1, h_hi_new)):
                            ps = scatter_psum(h)
                            epilogue(ps, h, h_new)

                    if layer < N_LAYERS - 1 and b >= GEMM_DLY:
                        emit_gemm_block(layer + 1, b - GEMM_DLY, h_lo_new, h_hi_new)
                    elif layer == N_LAYERS - 1:
                        k_ready = (b - 4 - 3) // 4   # blocks 4k..4k+3 done, +4 delay
                        for k in range(0, k_ready + 1):
                            emit_fc_chunk(k)
                if layer < N_LAYERS - 1:
                    for b in range(max(0, n_blk - GEMM_DLY), n_blk):
                        emit_gemm_block(layer + 1, b, h_lo_new, h_hi_new)
                h_lo, h_hi = h_lo_new, h_hi_new

            # ---- FC + sigmoid: flush chunks not emitted inline ----
            for k in range((n_loc + 511) // 512):
                emit_fc_chunk(k)
            nc.sync.dma_start(out=out_ext[:, :], in_=osb[:])

    mybir.codegen_inst_isa_subclasses(nc)
    _split_multiwaits(nc)
    return nc


def kernel(**inputs):
    global LAST_RESULTS, LAST_NC
    x = np.asarray(inputs["x"], dtype=np.float32)
    edge_index = np.asarray(inputs["edge_index"])
    n_nodes, feat = x.shape
    hid = np.asarray(inputs["W1"]).shape[1]

    cores, nlo, nhi, NCL, NCH, n_loc, n_blk, s_all = _prep_host(x, edge_index, n_nodes)
    x_tbl = (s_all[:, None] * x).astype(BF16)   # pre-scaled input table, same on all cores

    # BN affine folding: z = (agg + b - m) * a + be,  a = g * rsqrt(v + eps)
    aff = np.zeros((P, 12), np.float32)
    for i in (1, 2, 3):
        g = np.asarray(inputs[f"g{i}"], np.float32)
        be = np.asarray(inputs[f"be{i}"], np.float32)
        m = np.asarray(inputs[f"m{i}"], np.float32)
        v = np.asarray(inputs[f"v{i}"], np.float32)
        b = np.asarray(inputs[f"b{i}"], np.float32)
        a = g / np.sqrt(v + BN_EPS)
        cc = (b - m) * a + be
        L = i - 1
        aff[:, 4 * L + 0] = a[0:P]
        aff[:, 4 * L + 1] = a[P:2 * P]
        aff[:, 4 * L + 2] = cc[0:P]
        aff[:, 4 * L + 3] = cc[P:2 * P]

    wfc_np = np.zeros((P, 2), dtype=BF16)
    Wfc = np.asarray(inputs["Wfc"], np.float32)
    wfc_np[:, 0] = Wfc[0:P, 0].astype(BF16)
    wfc_np[:, 1] = Wfc[P:2 * P, 0].astype(BF16)
    bfc_val = float(np.asarray(inputs["bfc"]).reshape(-1)[0])
    iota_np = np.tile(np.arange(P, dtype=np.float32).astype(BF16)[None, :], (P, 1))

    nc = _build(n_nodes, n_loc, n_blk, nlo, nhi, NCL, NCH, feat, hid, bfc_val)

    in_maps = []
    for c in range(NCORES):
        d = cores[c]
        in_maps.append({
            "x_tbl": x_tbl,
            "W1": np.asarray(inputs["W1"], np.float32).astype(BF16),
            "W2": np.asarray(inputs["W2"], np.float32).astype(BF16),
            "W3": np.asarray(inputs["W3"], np.float32).astype(BF16),
            "wfc": wfc_np, "aff": aff,
            "s_col": d["s_col"], "s_bcast": d["s_bcast"], "iota": iota_np,
            "gidx_lo": d["gidx_lo"], "gidx_hi": d["gidx_hi"],
            "dstid_lo": d["dstid_lo"], "dstid_hi": d["dstid_hi"],
        })

    res = run_bass_kernel_spmd(nc, in_maps, core_ids=list(range(NCORES)))
    LAST_RESULTS = res
    globals()["LAST_NC"] = nc
    out = np.concatenate([res.results[c]["out"].reshape(-1) for c in range(NCORES)])
    return out.reshape(-1, 1).astype(np.float32)

